# revision 1
# baseline (speedup 1.0000x reference)
"""
Multi-head attention (B=2, S=2048, D=1024, H=16, hd=64) on 8 TRN2 NeuronCores.

Sharding: tensor-parallel over (batch, head-group).
  core = b*4 + g   (b in {0,1}, g in {0..3})  owns batch b, heads 4g..4g+3.

Per-core on-device pipeline (all matmuls in float32r at full PE rate):
  1. qT/kT = (Wqk_local).T-style projection:  psum <- wqk[kslice].T @ xT[kslice]
     -> qkT sbuf [4 ptiles x 2048]  (ptiles 0,1 = qT halves; 2,3 = kT halves)
     bias added on psum->sbuf drain (per-partition tensor_scalar_add).
  2. V_ext natural-layout projection: psum <- xT[kslice, rowtile].T @ wv_ext
     wv_ext has a zero column appended per head; on drain the whole tile is
     multiplied by the key-padding mask (per-partition scalar) and the zero
     columns are then overwritten with the mask itself.  The mask column
     rides the ctx matmul to produce the softmax denominators for free.
  3. Attention per (head-pair p, q-chunk c): scoresT[j, q] for both heads of
     the pair via row-packed K=64 matmuls (head A on partitions 0-63, head B
     on 64-127), one ACT exp over the pair's [128, 1024] psum, ctx
     accumulation ctxT[65, 512] over 16 j-tiles (row 64 = denominators).
  4. Normalization: denominators -> reciprocal (reshaped to [128, 64] for
     lane parallelism) -> partition-broadcast -> elementwise multiply.
  5. Output projection into psum, DMA straight to DRAM as a PARTIAL result
     (sum over this core's 4 heads only, no bias).

Host side: out[b] = sum of the 4 partials of batch b + (b_proj + b_v @ W_proj),
using softmax rows summing to 1 to fold the V bias into a constant vector.
"""

import ml_dtypes
import numpy as np

BF16 = ml_dtypes.bfloat16

B, S, D = 2, 2048, 1024
H, HD = 16, 64
NCORES = 8
HEADS_PER_CORE = 4  # 2 pairs
KSLICES = D // 128  # 8
QCHUNK = 512
NQC = S // QCHUNK  # 4
JT = S // 128  # 16 j tiles
RT = S // 128  # 16 row tiles
VW = HD + 1  # 65: v columns + mask column
VEXTW = HEADS_PER_CORE * VW  # 260

_cache = {}


def _build_program():
    import concourse.bass as bass
    import concourse.tile as tile
    from concourse import bacc, mybir

    f32 = mybir.dt.float32
    f32r = mybir.dt.float32r
    bf16 = mybir.dt.bfloat16
    Exp = mybir.ActivationFunctionType.Exp

    nc = bacc.Bacc(
        "TRN2",
        target_bir_lowering=False,
        debug=False,
        num_devices=NCORES,
        enable_partition_id=False,
    )

    xT_d = nc.dram_tensor("xT", [D, S], bf16, kind="ExternalInput").ap()
    wqk_d = nc.dram_tensor("wqk", [D, 512], bf16, kind="ExternalInput").ap()
    bqk_d = nc.dram_tensor("bqk", [128, 4], f32, kind="ExternalInput").ap()
    wv_d = nc.dram_tensor("wv", [D, VEXTW], bf16, kind="ExternalInput").ap()
    wp_d = nc.dram_tensor("wp", [256, D], bf16, kind="ExternalInput").ap()
    maskf_d = nc.dram_tensor("maskf", [128, RT], f32, kind="ExternalInput").ap()
    ones64_d = nc.dram_tensor("ones64", [128, 64], f32r, kind="ExternalInput").ap()
    out_d = nc.dram_tensor("out", [S, D], f32, kind="ExternalOutput").ap()

    def mm(out, lhsT, rhs, **kw):
        nc.tensor.matmul(out, lhsT, rhs, **kw)

    with tile.TileContext(nc) as tc:
        with tc.tile_pool(name="persist", bufs=1) as pp:
            qkT = pp.tile([128, 4 * S], bf16, tag="qkT")
            vext = pp.tile([128, RT * VEXTW], bf16, tag="vext")
            wp_sb = pp.tile([128, 2 * D], bf16, tag="wp")
            maskf = pp.tile([128, RT], f32, tag="maskf")
            bqk = pp.tile([128, 4], f32, tag="bqk")
            ones4 = pp.tile([128, 4], f32, tag="ones4")
            ctxT = pp.tile([128, 2 * S], bf16, tag="ctxT")
            ones64 = pp.tile([128, 64], f32r, tag="ones64")
            # head h's softmax denominators live at partition 32h (engine ops
            # require start partition in {0,32,64,96})
            sums_fl = pp.tile([128, S], f32, tag="sums_fl")
            recip_fl = pp.tile([128, S], f32r, tag="recip_fl")
            sums_rs = pp.tile([128, 64], f32, tag="sums_rs")
            recip_rs = pp.tile([128, 64], f32r, tag="recip_rs")
            xT = pp.tile([128, KSLICES * S], bf16, tag="xT")
            wqk = pp.tile([128, KSLICES * 512], bf16, tag="wqk")
            wv = pp.tile([128, KSLICES * VEXTW], bf16, tag="wv")

            nc.sync.dma_start(maskf[:], maskf_d[:])
            nc.sync.dma_start(bqk[:], bqk_d[:])
            for p in range(2):
                nc.sync.dma_start(
                    wp_sb[:, p * D : (p + 1) * D], wp_d[p * 128 : (p + 1) * 128, :]
                )
            nc.gpsimd.memset(ones4[:], 1.0)
            nc.sync.dma_start(ones64[:], ones64_d[:])
            for k in range(KSLICES):
                nc.sync.dma_start(
                    xT[:, k * S : (k + 1) * S], xT_d[k * 128 : (k + 1) * 128, :]
                )
                nc.sync.dma_start(
                    wqk[:, k * 512 : (k + 1) * 512], wqk_d[k * 128 : (k + 1) * 128, :]
                )
                nc.sync.dma_start(
                    wv[:, k * VEXTW : (k + 1) * VEXTW],
                    wv_d[k * 128 : (k + 1) * 128, :],
                )

            with (
                tc.tile_pool(name="pj", bufs=1, space="PSUM") as pj,
                tc.tile_pool(name="sc", bufs=1, space="PSUM") as scp,
                tc.tile_pool(name="cx", bufs=2, space="PSUM") as cxp,
                tc.tile_pool(name="ep", bufs=3) as ep,
            ):
                for p in range(2):  # head pair; qkv_p+1 fills PE gaps of attn_p
                    hA, hB = 2 * p, 2 * p + 1
                    # ---- qkv for this pair ----
                    for pt in (p, 2 + p):  # qT ptile p, kT ptile 2+p
                        for c in range(NQC):
                            ps = pj.tile([128, QCHUNK], f32, tag="pjqk")
                            for k in range(KSLICES):
                                mm(
                                    ps[:],
                                    wqk[:, k * 512 + pt * 128 : k * 512 + (pt + 1) * 128],
                                    xT[:, k * S + c * QCHUNK : k * S + (c + 1) * QCHUNK],
                                    start=(k == 0),
                                    stop=(k == KSLICES - 1),
                                )
                            nc.vector.tensor_scalar_add(
                                qkT[:, pt * S + c * QCHUNK : pt * S + (c + 1) * QCHUNK],
                                ps[:],
                                bqk[:, pt : pt + 1],
                            )
                    for t in range(RT):
                        ps = pj.tile([128, 2 * VW], f32, tag="pjv")
                        for k in range(KSLICES):
                            mm(
                                ps[:],
                                xT[:, k * S + t * 128 : k * S + (t + 1) * 128],
                                wv[:, k * VEXTW + p * 2 * VW : k * VEXTW + (p + 1) * 2 * VW],
                                start=(k == 0),
                                stop=(k == KSLICES - 1),
                            )
                        nc.vector.tensor_scalar_mul(
                            vext[:, t * VEXTW + p * 2 * VW : t * VEXTW + (p + 1) * 2 * VW],
                            ps[:],
                            maskf[:, t : t + 1],
                        )
                        mcols = vext[
                            :, t * VEXTW + p * 2 * VW : t * VEXTW + (p + 1) * 2 * VW
                        ].rearrange("p (h w) -> p h w", w=VW)[:, :, HD]
                        nc.vector.tensor_scalar_mul(
                            mcols, ones4[:, 0:2], maskf[:, t : t + 1]
                        )

                    # ---- attention for this pair ----
                    for c in range(NQC):
                        ctxA = cxp.tile([VW, QCHUNK], f32, tag="ctx")
                        ctxB = cxp.tile([VW, QCHUNK], f32, tag="ctx")
                        for jt2 in range(JT // 2):  # two j-tiles per exp round
                            sc = scp.tile([128, 4 * QCHUNK], f32, tag="sc")
                            for half, (lo, hi) in enumerate(((0, 64), (64, 128))):
                                for j01 in range(2):
                                    jt = 2 * jt2 + j01
                                    mm(
                                        sc[
                                            :,
                                            (2 * j01 + half) * QCHUNK : (2 * j01 + half + 1) * QCHUNK,
                                        ],
                                        qkT[lo:hi, (2 + p) * S + jt * 128 : (2 + p) * S + (jt + 1) * 128],
                                        qkT[lo:hi, p * S + c * QCHUNK : p * S + (c + 1) * QCHUNK],
                                        start=True,
                                        stop=True,
                                    )
                            e = ep.tile([128, 4 * QCHUNK], bf16, tag="e")
                            nc.scalar.activation(e[:], sc[:], Exp, scale=0.125)
                            for ctx_ps, h, half in ((ctxA, hA, 0), (ctxB, hB, 1)):
                                for j01 in range(2):
                                    jt = 2 * jt2 + j01
                                    mm(
                                        ctx_ps[:],
                                        vext[:, jt * VEXTW + h * VW : jt * VEXTW + (h + 1) * VW],
                                        e[:, (2 * j01 + half) * QCHUNK : (2 * j01 + half + 1) * QCHUNK],
                                        start=(jt == 0),
                                        stop=(jt == JT - 1),
                                        skip_group_check=True,
                                    )
                        # drain: ctx rows 0-63 -> ctxT, row 64 -> sums
                        for ctx_ps, h, half in ((ctxA, hA, 0), (ctxB, hB, 1)):
                            nc.vector.tensor_copy(
                                ctxT[
                                    half * HD : (half + 1) * HD,
                                    p * S + c * QCHUNK : p * S + (c + 1) * QCHUNK,
                                ],
                                ctx_ps[0:HD, :],
                            )
                            nc.vector.tensor_copy(
                                sums_fl[32 * h : 32 * h + 1, c * QCHUNK : (c + 1) * QCHUNK],
                                ctx_ps[HD : HD + 1, :],
                            )

            # ---------------- normalize ----------------
            # reciprocal is ~8 cyc/elem/lane; gather the 16 live [1,512] sums
            # rows into a dense [128,64] tile so all lanes work (13us -> 0.5us)
            nc.gpsimd.dma_start(
                sums_rs[:],
                sums_fl.rearrange("(a b) f -> a b f", b=32)[:, 0, :],
            )
            with nc.allow_low_precision(reason="f32r rounding of softmax recip"):
                nc.vector.reciprocal(recip_rs[:], sums_rs[:])
            nc.gpsimd.dma_start(
                recip_fl.rearrange("(a b) f -> a b f", b=32)[:, 0, :],
                recip_rs[:],
            )
            # recipb = ones64.T @ recip_row via K=1 matmuls (PE partition
            # broadcast: gpsimd partition_broadcast is broken on HW)
            with (
                tc.tile_pool(name="rb", bufs=1, space="PSUM") as rbp,
                tc.tile_pool(name="po", bufs=4, space="PSUM") as po,
                tc.tile_pool(name="ob", bufs=4) as ob,
            ):
                for p in range(2):
                    for half in range(2):
                        h = 2 * p + half
                        rb = rbp.tile([HD, S], f32, tag="rb")
                        for c in range(NQC):
                            mm(
                                rb[:, c * QCHUNK : (c + 1) * QCHUNK],
                                ones64[32 * h : 32 * h + 1, :],
                                recip_fl[
                                    32 * h : 32 * h + 1,
                                    c * QCHUNK : (c + 1) * QCHUNK,
                                ],
                                start=True,
                                stop=True,
                                # auto-derive caps at 64; row group 3 is explicit
                                tile_position=(32 * h, 0) if h == 3 else None,
                            )
                        sl = ctxT[
                            half * HD : (half + 1) * HD, p * S : (p + 1) * S
                        ]
                        nc.vector.tensor_mul(sl, sl, rb[:])

                # ---------------- output projection ----------------
                for qt in range(S // 128):
                    for oc in range(2):
                        ps = po.tile([128, QCHUNK], f32, tag="po")
                        for p in range(2):
                            mm(
                                ps[:],
                                ctxT[:, p * S + qt * 128 : p * S + (qt + 1) * 128],
                                wp_sb[:, p * D + oc * QCHUNK : p * D + (oc + 1) * QCHUNK],
                                start=(p == 0),
                                stop=(p == 1),
                            )
                        o = ob.tile([128, QCHUNK], f32, tag="o")
                        nc.vector.tensor_copy(o[:], ps[:])
                        nc.sync.dma_start(
                            out_d[
                                qt * 128 : (qt + 1) * 128,
                                oc * QCHUNK : (oc + 1) * QCHUNK,
                            ],
                            o[:],
                        )

    nc.compile()
    return nc


def get_program():
    if "nc" not in _cache:
        _cache["nc"] = _build_program()
    return _cache["nc"]


def make_in_maps(x, mask, W_qkv, b_qkv, W_proj):
    """Build the 8 per-core input maps (host-side sharding)."""
    x = np.asarray(x, dtype=np.float32)
    mask = np.asarray(mask)
    W_qkv = np.asarray(W_qkv, dtype=np.float32)
    b_qkv = np.asarray(b_qkv, dtype=np.float32)
    W_proj = np.asarray(W_proj, dtype=np.float32)

    in_maps = []
    for core in range(NCORES):
        b, g = divmod(core, 4)
        qc = slice(256 * g, 256 * (g + 1))  # q cols for heads 4g..4g+3
        kc = slice(D + 256 * g, D + 256 * (g + 1))
        vc = slice(2 * D + 256 * g, 2 * D + 256 * (g + 1))

        xT = np.ascontiguousarray(x[b].T).astype(BF16)

        wqk = np.concatenate([W_qkv[:, qc], W_qkv[:, kc]], axis=1)
        wqk = np.ascontiguousarray(wqk).astype(BF16)

        bq = b_qkv[qc]
        bk = b_qkv[kc]
        bqk = np.stack(
            [bq[:128], bq[128:], bk[:128], bk[128:]], axis=1
        )  # [128, 4]
        bqk = np.ascontiguousarray(bqk)

        wv_ext = np.zeros((D, VEXTW), dtype=np.float32)
        for h in range(HEADS_PER_CORE):
            wv_ext[:, h * VW : h * VW + HD] = W_qkv[:, 2 * D + 256 * g + HD * h : 2 * D + 256 * g + HD * (h + 1)]

        wp = np.ascontiguousarray(W_proj[256 * g : 256 * (g + 1), :]).astype(BF16)

        maskf = np.ascontiguousarray(
            mask[b].astype(np.float32).reshape(RT, 128).T
        )  # [128, RT] col t = rowtile t

        in_maps.append(
            {
                "xT": xT,
                "wqk": wqk,
                "bqk": bqk,
                "wv": wv_ext.astype(BF16),
                "wp": wp,
                "maskf": maskf,
                "ones64": np.ones((128, 64), dtype=np.float32),
            }
        )
    return in_maps


def kernel(x, mask, W_qkv, b_qkv, W_proj, b_proj, _trace=False):
    from concourse import bass_utils

    nc = get_program()
    in_maps = make_in_maps(x, mask, W_qkv, b_qkv, W_proj)

    res = bass_utils.run_bass_kernel_spmd(
        nc, in_maps, list(range(NCORES)), trace=_trace
    )
    _cache["last_results"] = res

    b_qkv = np.asarray(b_qkv, dtype=np.float32)
    W_proj = np.asarray(W_proj, dtype=np.float32)
    bias_full = np.asarray(b_proj, dtype=np.float32) + b_qkv[2 * D :] @ W_proj

    out = np.empty((B, S, D), dtype=np.float32)
    for b in range(B):
        acc = bias_full[None, :].repeat(S, axis=0).astype(np.float32)
        for g in range(4):
            acc = acc + res.results[b * 4 + g]["out"]
        out[b] = acc
    return out



# revision 3
# speedup vs baseline: 1.5695x; 1.5695x over previous
"""
Multi-head attention (B=2, S=2048, D=1024, H=16, hd=64) on 8 TRN2 NeuronCores.

Sharding: tensor-parallel over (batch, head-group).
  core = b*4 + g   (b in {0,1}, g in {0..3})  owns batch b, heads 4g..4g+3.

v2 pipeline design (vs baseline): the scalar engine (exp) is the roofline at
~1.15us per [128,1024] round; everything else is scheduled to hide under it.

  - Rounds of 1 j-tile x 1 head-pair: scores psum [128,1024] double-buffered
    (2x2 banks) so exp(r) overlaps scores(r+1); exp runs back-to-back.
  - ctx psum [65,512] per head accumulates over the 16 j-tiles of a
    (chunk, pair); row 64 rides the matmul as the softmax denominator
    (vext mask-column trick, unchanged from baseline).
  - Chunk-outer loop: after both pairs finish chunk c, normalize + output
    projection + DMA for chunk c are emitted as fillers into chunk c+1's
    rounds (no serial tail except chunk 3).
  - Projection chains (qT/kT per (ptile,chunk), vext per row-tile) are
    emitted JIT via a filler queue: Tile's scheduler priority == emission
    order, so filler closures are popped between rounds, with forced
    emission of any chain a round depends on.
  - PSUM budget: sc 2x2 + ctx/rb 2 + proj 2 = 8 banks.
  - Output partials stored bf16 (halves DMA; host sums in f32).
"""

import ml_dtypes
import numpy as np

BF16 = ml_dtypes.bfloat16

B, S, D = 2, 2048, 1024
H, HD = 16, 64
NCORES = 8
HEADS_PER_CORE = 4
KSLICES = D // 128  # 8
QCHUNK = 512
NQC = S // QCHUNK  # 4
JT = S // 128  # 16 j tiles
RT = S // 128  # 16 row tiles
VW = HD + 1  # 65: v columns + mask column
VEXTW = HEADS_PER_CORE * VW  # 260

_cache = {}


def _build_program():
    import concourse.bass as bass
    import concourse.tile as tile
    from concourse import bacc, mybir

    f32 = mybir.dt.float32
    f32r = mybir.dt.float32r
    bf16 = mybir.dt.bfloat16
    Exp = mybir.ActivationFunctionType.Exp

    nc = bacc.Bacc(
        "TRN2",
        target_bir_lowering=False,
        debug=False,
        num_devices=NCORES,
        enable_partition_id=False,
    )

    xT_d = nc.dram_tensor("xT", [D, S], bf16, kind="ExternalInput").ap()
    wqk_d = nc.dram_tensor("wqk", [D, 512], bf16, kind="ExternalInput").ap()
    bqk_d = nc.dram_tensor("bqk", [128, 4], f32, kind="ExternalInput").ap()
    wv_d = nc.dram_tensor("wv", [D, VEXTW], bf16, kind="ExternalInput").ap()
    wp_d = nc.dram_tensor("wp", [256, D], bf16, kind="ExternalInput").ap()
    maskf_d = nc.dram_tensor("maskf", [128, RT], f32, kind="ExternalInput").ap()
    ones64_d = nc.dram_tensor("ones64", [128, 64], f32r, kind="ExternalInput").ap()
    out_d = nc.dram_tensor("out", [S, D], bf16, kind="ExternalOutput").ap()

    def mm(out, lhsT, rhs, **kw):
        nc.tensor.matmul(out, lhsT, rhs, **kw)

    with tile.TileContext(nc) as tc:
        with tc.tile_pool(name="persist", bufs=1) as pp:
            qkT = pp.tile([128, 4 * S], bf16, tag="qkT")
            vext = pp.tile([128, RT * VEXTW], bf16, tag="vext")
            wp_sb = pp.tile([128, 2 * D], bf16, tag="wp")
            maskf = pp.tile([128, RT], f32, tag="maskf")
            bqk = pp.tile([128, 4], f32, tag="bqk")
            ones4 = pp.tile([128, 4], f32, tag="ones4")
            ctxT = pp.tile([128, 2 * S], bf16, tag="ctxT")
            ones64 = pp.tile([128, 64], f32r, tag="ones64")
            sums_fl = pp.tile([128, S], f32, tag="sums_fl")
            recip_fl = pp.tile([128, S], f32r, tag="recip_fl")
            sums_rs = pp.tile([128, 64], f32, tag="sums_rs")
            recip_rs = pp.tile([128, 64], f32r, tag="recip_rs")
            xT = pp.tile([128, KSLICES * S], bf16, tag="xT")
            wqk = pp.tile([128, KSLICES * 512], bf16, tag="wqk")
            wv = pp.tile([128, KSLICES * VEXTW], bf16, tag="wv")

            # ---- input DMAs, ordered by first use; xT split into column
            # halves so chunk-0/strip-0 chains unblock at ~60% of the load ----
            nc.gpsimd.memset(ones4[:], 1.0)
            for k in range(KSLICES):
                nc.sync.dma_start(
                    wqk[:, k * 512 : (k + 1) * 512], wqk_d[k * 128 : (k + 1) * 128, :]
                )
            for k in range(KSLICES):
                nc.sync.dma_start(
                    xT[:, k * S : k * S + 1024],
                    xT_d[k * 128 : (k + 1) * 128, 0:1024],
                )
            nc.sync.dma_start(bqk[:], bqk_d[:])
            nc.sync.dma_start(maskf[:], maskf_d[:])
            for k in range(KSLICES):
                nc.sync.dma_start(
                    xT[:, k * S + 1024 : (k + 1) * S],
                    xT_d[k * 128 : (k + 1) * 128, 1024:2048],
                )
                nc.sync.dma_start(
                    wv[:, k * VEXTW : (k + 1) * VEXTW],
                    wv_d[k * 128 : (k + 1) * 128, :],
                )
            nc.sync.dma_start(ones64[:], ones64_d[:])
            for p in range(2):
                nc.sync.dma_start(
                    wp_sb[:, p * D : (p + 1) * D], wp_d[p * 128 : (p + 1) * 128, :]
                )

            with (
                tc.tile_pool(name="pj", bufs=2, space="PSUM") as pj,
                tc.tile_pool(name="sc", bufs=2, space="PSUM") as scp,
                tc.tile_pool(name="cx", bufs=2, space="PSUM") as cxp,
                tc.tile_pool(name="ep", bufs=3) as ep,
                tc.tile_pool(name="ob", bufs=2) as ob,
            ):
                # ---------- filler machinery ----------
                # Each chain is a list of closures; emission order == Tile
                # scheduling priority, so chains are dribbled between rounds.
                filler_q = []  # list of (key, closure)
                done = set()  # chain keys fully emitted
                emitted_keys = set()

                def make_qk_chain(pt, c):
                    st = {}
                    steps = []
                    for k in range(KSLICES):
                        def step(k=k, pt=pt, c=c):
                            if k == 0:
                                st["ps"] = pj.tile([128, QCHUNK], f32, tag="pj", name=f"pjqk_{pt}_{c}")
                            mm(
                                st["ps"][:],
                                wqk[:, k * 512 + pt * 128 : k * 512 + (pt + 1) * 128],
                                xT[:, k * S + c * QCHUNK : k * S + (c + 1) * QCHUNK],
                                start=(k == 0),
                                stop=(k == KSLICES - 1),
                            )
                        steps.append(step)
                    def drain(pt=pt, c=c):
                        nc.vector.tensor_scalar_add(
                            qkT[:, pt * S + c * QCHUNK : pt * S + (c + 1) * QCHUNK],
                            st["ps"][:],
                            bqk[:, pt : pt + 1],
                        )
                    steps.append(drain)
                    return steps

                def make_v_chain(t):
                    st = {}
                    steps = []
                    for k in range(KSLICES):
                        def step(k=k, t=t):
                            if k == 0:
                                st["ps"] = pj.tile([128, VEXTW], f32, tag="pj", name=f"pjv_{t}")
                            mm(
                                st["ps"][:],
                                xT[:, k * S + t * 128 : k * S + (t + 1) * 128],
                                wv[:, k * VEXTW : (k + 1) * VEXTW],
                                start=(k == 0),
                                stop=(k == KSLICES - 1),
                            )
                        steps.append(step)
                    def drain(t=t):
                        sl = vext[:, t * VEXTW : (t + 1) * VEXTW]
                        nc.vector.tensor_scalar_mul(sl, st["ps"][:], maskf[:, t : t + 1])
                        mcols = sl.rearrange("p (h w) -> p h w", w=VW)[:, :, HD]
                        nc.vector.tensor_scalar_mul(
                            mcols, ones4[:, 0:4], maskf[:, t : t + 1]
                        )
                    steps.append(drain)
                    return steps

                def add_chain(key, steps):
                    emitted_keys.add(key)
                    for s in steps:
                        filler_q.append((key, s))

                # chains ordered by first use:
                # round index = c*32 + p*16 + jt
                chain_first_use = []
                for p in range(2):
                    for c in range(NQC):
                        chain_first_use.append((c * 32 + p * 16, ("qk", p, c)))
                    for s_ in range(4):
                        chain_first_use.append((p * 16 + 4 * s_, ("qk", 2 + p, s_)))
                for t in range(RT):
                    chain_first_use.append((t, ("vext", t)))
                chain_first_use.sort(key=lambda x: x[0])
                for _, key in chain_first_use:
                    if key[0] == "qk":
                        add_chain(key, make_qk_chain(key[1], key[2]))
                    else:
                        add_chain(key, make_v_chain(key[1]))

                def need(key):
                    if key in done:
                        return
                    while key not in done:
                        k2, closure = filler_q.pop(0)
                        closure()
                        if not filler_q or filler_q[0][0] != k2:
                            done.add(k2)

                def pop_fillers(n):
                    for _ in range(n):
                        if not filler_q:
                            return
                        k2, closure = filler_q.pop(0)
                        closure()
                        if not filler_q or filler_q[0][0] != k2:
                            done.add(k2)

                # ---------- norm + output projection fillers (per chunk) ----------
                def add_norm_outproj(c):
                    key = ("norm", c)
                    steps = []
                    st = {}
                    def gather(c=c):
                        nc.gpsimd.dma_start(
                            sums_rs[:, c * 16 : (c + 1) * 16],
                            sums_fl[:, c * QCHUNK : (c + 1) * QCHUNK].rearrange(
                                "(a b) f -> a b f", b=32
                            )[:, 0, :],
                        )
                    steps.append(gather)
                    def recip(c=c):
                        with nc.allow_low_precision(reason="f32r softmax recip"):
                            nc.vector.reciprocal(
                                recip_rs[:, c * 16 : (c + 1) * 16],
                                sums_rs[:, c * 16 : (c + 1) * 16],
                            )
                    steps.append(recip)
                    def scatter(c=c):
                        nc.gpsimd.dma_start(
                            recip_fl[:, c * QCHUNK : (c + 1) * QCHUNK].rearrange(
                                "(a b) f -> a b f", b=32
                            )[:, 0, :],
                            recip_rs[:, c * 16 : (c + 1) * 16],
                        )
                    steps.append(scatter)
                    for h in range(4):
                        def bmul(h=h, c=c):
                            rb = cxp.tile([HD, QCHUNK], f32, tag="ctx")
                            mm(
                                rb[:],
                                ones64[32 * h : 32 * h + 1, :],
                                recip_fl[
                                    32 * h : 32 * h + 1,
                                    c * QCHUNK : (c + 1) * QCHUNK,
                                ],
                                start=True,
                                stop=True,
                                tile_position=(32 * h, 0) if h == 3 else None,
                            )
                            p, half = h // 2, h % 2
                            sl = ctxT[
                                half * HD : (half + 1) * HD,
                                p * S + c * QCHUNK : p * S + (c + 1) * QCHUNK,
                            ]
                            nc.vector.tensor_mul(sl, sl, rb[:])
                        steps.append(bmul)
                    for qt in range(4 * c, 4 * c + 4):
                        for oc in range(2):
                            def proj(qt=qt, oc=oc):
                                ps = pj.tile([128, QCHUNK], f32, tag="pj")
                                for p in range(2):
                                    mm(
                                        ps[:],
                                        ctxT[:, p * S + qt * 128 : p * S + (qt + 1) * 128],
                                        wp_sb[:, p * D + oc * QCHUNK : p * D + (oc + 1) * QCHUNK],
                                        start=(p == 0),
                                        stop=(p == 1),
                                    )
                                o = ob.tile([128, QCHUNK], bf16, tag="o")
                                nc.vector.tensor_copy(o[:], ps[:])
                                nc.sync.dma_start(
                                    out_d[
                                        qt * 128 : (qt + 1) * 128,
                                        oc * QCHUNK : (oc + 1) * QCHUNK,
                                    ],
                                    o[:],
                                )
                            steps.append(proj)
                    add_chain(key, steps)

                # ---------- main round loop ----------
                for c in range(NQC):
                    for p in range(2):
                        need(("qk", p, c))
                        ctxA = cxp.tile([VW, QCHUNK], f32, tag="ctx")
                        ctxB = cxp.tile([VW, QCHUNK], f32, tag="ctx")
                        for jt in range(JT):
                            need(("qk", 2 + p, jt // 4))
                            need(("vext", jt))
                            sc = scp.tile([128, 2 * QCHUNK], f32, tag="sc")
                            for half, (lo, hi) in enumerate(((0, 64), (64, 128))):
                                mm(
                                    sc[:, half * QCHUNK : (half + 1) * QCHUNK],
                                    qkT[
                                        lo:hi,
                                        (2 + p) * S + jt * 128 : (2 + p) * S + (jt + 1) * 128,
                                    ],
                                    qkT[lo:hi, p * S + c * QCHUNK : p * S + (c + 1) * QCHUNK],
                                    start=True,
                                    stop=True,
                                )
                            e = ep.tile([128, 2 * QCHUNK], bf16, tag="e")
                            nc.scalar.activation(e[:], sc[:], Exp, scale=0.125)
                            for ctx_ps, h, half in (
                                (ctxA, 2 * p, 0),
                                (ctxB, 2 * p + 1, 1),
                            ):
                                mm(
                                    ctx_ps[:],
                                    vext[:, jt * VEXTW + h * VW : jt * VEXTW + (h + 1) * VW],
                                    e[:, half * QCHUNK : (half + 1) * QCHUNK],
                                    start=(jt == 0),
                                    stop=(jt == JT - 1),
                                    skip_group_check=True,
                                )
                            pop_fillers(3)
                        # drain ctx rows 0-63 -> ctxT, row 64 -> denominators
                        for ctx_ps, h, half in ((ctxA, 2 * p, 0), (ctxB, 2 * p + 1, 1)):
                            nc.vector.tensor_copy(
                                ctxT[
                                    half * HD : (half + 1) * HD,
                                    p * S + c * QCHUNK : p * S + (c + 1) * QCHUNK,
                                ],
                                ctx_ps[0:HD, :],
                            )
                            nc.vector.tensor_copy(
                                sums_fl[
                                    32 * h : 32 * h + 1, c * QCHUNK : (c + 1) * QCHUNK
                                ],
                                ctx_ps[HD : HD + 1, :],
                            )
                    add_norm_outproj(c)
                # flush remaining fillers (chunk-3 norm/outproj and stragglers)
                pop_fillers(len(filler_q))

    nc.compile()
    return nc


def get_program():
    if "nc" not in _cache:
        _cache["nc"] = _build_program()
    return _cache["nc"]


def make_in_maps(x, mask, W_qkv, b_qkv, W_proj):
    """Build the 8 per-core input maps (host-side sharding)."""
    x = np.asarray(x, dtype=np.float32)
    mask = np.asarray(mask)
    W_qkv = np.asarray(W_qkv, dtype=np.float32)
    b_qkv = np.asarray(b_qkv, dtype=np.float32)
    W_proj = np.asarray(W_proj, dtype=np.float32)

    in_maps = []
    for core in range(NCORES):
        b, g = divmod(core, 4)
        qc = slice(256 * g, 256 * (g + 1))  # q cols for heads 4g..4g+3
        kc = slice(D + 256 * g, D + 256 * (g + 1))

        xT = np.ascontiguousarray(x[b].T).astype(BF16)

        wqk = np.concatenate([W_qkv[:, qc], W_qkv[:, kc]], axis=1)
        wqk = np.ascontiguousarray(wqk).astype(BF16)

        bq = b_qkv[qc]
        bk = b_qkv[kc]
        bqk = np.stack([bq[:128], bq[128:], bk[:128], bk[128:]], axis=1)  # [128, 4]
        bqk = np.ascontiguousarray(bqk)

        wv_ext = np.zeros((D, VEXTW), dtype=np.float32)
        for h in range(HEADS_PER_CORE):
            wv_ext[:, h * VW : h * VW + HD] = W_qkv[
                :, 2 * D + 256 * g + HD * h : 2 * D + 256 * g + HD * (h + 1)
            ]

        wp = np.ascontiguousarray(W_proj[256 * g : 256 * (g + 1), :]).astype(BF16)

        maskf = np.ascontiguousarray(
            mask[b].astype(np.float32).reshape(RT, 128).T
        )  # [128, RT] col t = rowtile t

        in_maps.append(
            {
                "xT": xT,
                "wqk": wqk,
                "bqk": bqk,
                "wv": wv_ext.astype(BF16),
                "wp": wp,
                "maskf": maskf,
                "ones64": np.ones((128, 64), dtype=np.float32),
            }
        )
    return in_maps


def kernel(x, mask, W_qkv, b_qkv, W_proj, b_proj, _trace=False):
    from concourse import bass_utils

    nc = get_program()
    in_maps = make_in_maps(x, mask, W_qkv, b_qkv, W_proj)

    res = bass_utils.run_bass_kernel_spmd(
        nc, in_maps, list(range(NCORES)), trace=_trace
    )
    _cache["last_results"] = res

    b_qkv = np.asarray(b_qkv, dtype=np.float32)
    W_proj = np.asarray(W_proj, dtype=np.float32)
    bias_full = np.asarray(b_proj, dtype=np.float32) + b_qkv[2 * D :] @ W_proj

    out = np.empty((B, S, D), dtype=np.float32)
    for b in range(B):
        acc = bias_full[None, :].repeat(S, axis=0).astype(np.float32)
        for g in range(4):
            acc = acc + np.asarray(res.results[b * 4 + g]["out"]).astype(np.float32)
        out[b] = acc
    return out


# revision 7
# speedup vs baseline: 1.6011x; 1.0201x over previous
"""
Multi-head attention (B=2, S=2048, D=1024, H=16, hd=64) on 8 TRN2 NeuronCores.

Sharding: tensor-parallel over (batch, head-group).
  core = b*4 + g   (b in {0,1}, g in {0..3})  owns batch b, heads 4g..4g+3.

v2 pipeline design (vs baseline): the scalar engine (exp) is the roofline at
~1.15us per [128,1024] round; everything else is scheduled to hide under it.

  - Rounds of 1 j-tile x 1 head-pair: scores psum [128,1024] double-buffered
    (2x2 banks) so exp(r) overlaps scores(r+1); exp runs back-to-back.
  - ctx psum [65,512] per head accumulates over the 16 j-tiles of a
    (chunk, pair); row 64 rides the matmul as the softmax denominator
    (vext mask-column trick, unchanged from baseline).
  - Chunk-outer loop: after both pairs finish chunk c, normalize + output
    projection + DMA for chunk c are emitted as fillers into chunk c+1's
    rounds (no serial tail except chunk 3).
  - Projection chains (qT/kT per (ptile,chunk), vext per row-tile) are
    emitted JIT via a filler queue: Tile's scheduler priority == emission
    order, so filler closures are popped between rounds, with forced
    emission of any chain a round depends on.
  - PSUM budget: sc 2x2 + ctx/rb 2 + proj 2 = 8 banks.
  - Output partials stored bf16 (halves DMA; host sums in f32).
"""

import ml_dtypes
import numpy as np

BF16 = ml_dtypes.bfloat16

B, S, D = 2, 2048, 1024
H, HD = 16, 64
NCORES = 8
HEADS_PER_CORE = 4
KSLICES = D // 128  # 8
QCHUNK = 512
NQC = S // QCHUNK  # 4
JT = S // 128  # 16 j tiles
RT = S // 128  # 16 row tiles
VW = HD + 1  # 65: v columns + mask column
VEXTW = HEADS_PER_CORE * VW  # 260

_cache = {}


def _build_program():
    import concourse.bass as bass
    import concourse.tile as tile
    from concourse import bacc, mybir

    f32 = mybir.dt.float32
    f32r = mybir.dt.float32r
    bf16 = mybir.dt.bfloat16
    Exp = mybir.ActivationFunctionType.Exp

    nc = bacc.Bacc(
        "TRN2",
        target_bir_lowering=False,
        debug=False,
        num_devices=NCORES,
        enable_partition_id=False,
    )

    xT_d = nc.dram_tensor("xT", [D, S], bf16, kind="ExternalInput").ap()
    wqk_d = nc.dram_tensor("wqk", [D, 512], bf16, kind="ExternalInput").ap()
    bqk_d = nc.dram_tensor("bqk", [128, 4], f32, kind="ExternalInput").ap()
    wv_d = nc.dram_tensor("wv", [D, VEXTW], bf16, kind="ExternalInput").ap()
    wp_d = nc.dram_tensor("wp", [256, D], bf16, kind="ExternalInput").ap()
    maskf_d = nc.dram_tensor("maskf", [128, RT], f32, kind="ExternalInput").ap()
    ones64_d = nc.dram_tensor("ones64", [128, 64], f32r, kind="ExternalInput").ap()
    out_d = nc.dram_tensor("out", [S, D], bf16, kind="ExternalOutput").ap()

    def mm(out, lhsT, rhs, **kw):
        nc.tensor.matmul(out, lhsT, rhs, **kw)

    with tile.TileContext(nc) as tc:
        with tc.tile_pool(name="persist", bufs=1) as pp:
            qkT = pp.tile([128, 4 * S], bf16, tag="qkT")
            vext = pp.tile([128, RT * VEXTW], bf16, tag="vext")
            wp_sb = pp.tile([128, 2 * D], bf16, tag="wp")
            maskf = pp.tile([128, RT], f32, tag="maskf")
            bqk = pp.tile([128, 4], f32, tag="bqk")
            ones4 = pp.tile([128, 4], f32, tag="ones4")
            ctxT = pp.tile([128, 2 * S], bf16, tag="ctxT")
            ones64 = pp.tile([128, 64], f32r, tag="ones64")
            sums_fl = pp.tile([128, S], f32, tag="sums_fl")
            recip_fl = pp.tile([128, S], f32r, tag="recip_fl")
            sums_rs = pp.tile([128, 64], f32, tag="sums_rs")
            recip_rs = pp.tile([128, 64], f32r, tag="recip_rs")
            xT = pp.tile([128, KSLICES * S], bf16, tag="xT")
            wqk = pp.tile([128, KSLICES * 512], bf16, tag="wqk")
            wv = pp.tile([128, KSLICES * VEXTW], bf16, tag="wv")

            # ---- input DMAs, ordered by first use; xT split into column
            # halves so chunk-0/strip-0 chains unblock at ~60% of the load ----
            nc.gpsimd.memset(ones4[:], 1.0)
            nc.gpsimd.memset(sums_fl[:], 1.0)
            for k in range(KSLICES):
                nc.sync.dma_start(
                    wqk[:, k * 512 : (k + 1) * 512], wqk_d[k * 128 : (k + 1) * 128, :]
                )
            for k in range(KSLICES):
                nc.sync.dma_start(
                    wv[:, k * VEXTW : (k + 1) * VEXTW],
                    wv_d[k * 128 : (k + 1) * 128, :],
                )
            nc.sync.dma_start(bqk[:], bqk_d[:])
            nc.sync.dma_start(maskf[:], maskf_d[:])
            for k in range(KSLICES):
                nc.sync.dma_start(
                    xT[:, k * S : k * S + 1024],
                    xT_d[k * 128 : (k + 1) * 128, 0:1024],
                )
            for k in range(KSLICES):
                nc.sync.dma_start(
                    xT[:, k * S + 1024 : (k + 1) * S],
                    xT_d[k * 128 : (k + 1) * 128, 1024:2048],
                )
            nc.sync.dma_start(ones64[:], ones64_d[:])
            for p in range(2):
                nc.sync.dma_start(
                    wp_sb[:, p * D : (p + 1) * D], wp_d[p * 128 : (p + 1) * 128, :]
                )

            with (
                tc.tile_pool(name="pj", bufs=2, space="PSUM") as pj,
                tc.tile_pool(name="sc", bufs=2, space="PSUM") as scp,
                tc.tile_pool(name="cx", bufs=2, space="PSUM") as cxp,
                tc.tile_pool(name="ep", bufs=3) as ep,
                tc.tile_pool(name="ob", bufs=2) as ob,
            ):
                # ---------- filler machinery ----------
                # Each chain is a list of closures; emission order == Tile
                # scheduling priority, so chains are dribbled between rounds.
                filler_q = []  # list of (key, closure)
                done = set()  # chain keys fully emitted
                emitted_keys = set()

                def make_qk_chain(pt, c):
                    st = {}
                    steps = []
                    for k in range(KSLICES):
                        def step(k=k, pt=pt, c=c):
                            if k == 0:
                                st["ps"] = pj.tile([128, QCHUNK], f32, tag="pj", name=f"pjqk_{pt}_{c}")
                            mm(
                                st["ps"][:],
                                wqk[:, k * 512 + pt * 128 : k * 512 + (pt + 1) * 128],
                                xT[:, k * S + c * QCHUNK : k * S + (c + 1) * QCHUNK],
                                start=(k == 0),
                                stop=(k == KSLICES - 1),
                            )
                        steps.append(step)
                    def drain(pt=pt, c=c):
                        nc.vector.tensor_scalar_add(
                            qkT[:, pt * S + c * QCHUNK : pt * S + (c + 1) * QCHUNK],
                            st["ps"][:],
                            bqk[:, pt : pt + 1],
                        )
                    steps.append(drain)
                    return steps

                def make_v_chain(t):
                    st = {}
                    steps = []
                    for k in range(KSLICES):
                        def step(k=k, t=t):
                            if k == 0:
                                st["ps"] = pj.tile([128, VEXTW], f32, tag="pj", name=f"pjv_{t}")
                            mm(
                                st["ps"][:],
                                xT[:, k * S + t * 128 : k * S + (t + 1) * 128],
                                wv[:, k * VEXTW : (k + 1) * VEXTW],
                                start=(k == 0),
                                stop=(k == KSLICES - 1),
                            )
                        steps.append(step)
                    def drain(t=t):
                        sl = vext[:, t * VEXTW : (t + 1) * VEXTW]
                        nc.vector.tensor_scalar_mul(sl, st["ps"][:], maskf[:, t : t + 1])
                        mcols = sl.rearrange("p (h w) -> p h w", w=VW)[:, :, HD]
                        nc.vector.tensor_scalar_mul(
                            mcols, ones4[:, 0:4], maskf[:, t : t + 1]
                        )
                    steps.append(drain)
                    return steps

                def add_chain(key, steps):
                    emitted_keys.add(key)
                    for s in steps:
                        filler_q.append((key, s))

                # chains ordered by first use:
                # round index = c*32 + p*16 + jt
                chain_first_use = []
                for p in range(2):
                    for c in range(NQC):
                        chain_first_use.append((c * 32 + p * 16, ("qk", p, c)))
                    for s_ in range(4):
                        chain_first_use.append((p * 16 + 4 * s_, ("qk", 2 + p, s_)))
                for t in range(RT):
                    chain_first_use.append((t, ("vext", t)))
                chain_first_use.sort(key=lambda x: x[0])
                for _, key in chain_first_use:
                    if key[0] == "qk":
                        add_chain(key, make_qk_chain(key[1], key[2]))
                    else:
                        add_chain(key, make_v_chain(key[1]))

                def need(key):
                    if key in done:
                        return
                    while key not in done:
                        k2, closure = filler_q.pop(0)
                        closure()
                        if not filler_q or filler_q[0][0] != k2:
                            done.add(k2)

                def pop_fillers(n):
                    for _ in range(n):
                        if not filler_q:
                            return
                        k2, closure = filler_q.pop(0)
                        closure()
                        if not filler_q or filler_q[0][0] != k2:
                            done.add(k2)

                # ---------- norm + output projection fillers (per chunk) ----------
                def add_norm_pair(c, p):
                    """Normalize pair p's 2 heads of chunk c (overlaps other work)."""
                    key = ("norm", c, p)
                    steps = []
                    def gather(c=c, p=p):
                        nc.gpsimd.dma_start(
                            sums_rs[64 * p : 64 * p + 64, c * 16 : (c + 1) * 16],
                            sums_fl[
                                64 * p : 64 * p + 64, c * QCHUNK : (c + 1) * QCHUNK
                            ].rearrange("(a b) f -> a b f", b=32)[:, 0, :],
                        )
                    steps.append(gather)
                    def recip(c=c, p=p):
                        with nc.allow_low_precision(reason="f32r softmax recip"):
                            nc.vector.reciprocal(
                                recip_rs[64 * p : 64 * p + 64, c * 16 : (c + 1) * 16],
                                sums_rs[64 * p : 64 * p + 64, c * 16 : (c + 1) * 16],
                            )
                    steps.append(recip)
                    def scatter(c=c, p=p):
                        nc.gpsimd.dma_start(
                            recip_fl[
                                64 * p : 64 * p + 64, c * QCHUNK : (c + 1) * QCHUNK
                            ].rearrange("(a b) f -> a b f", b=32)[:, 0, :],
                            recip_rs[64 * p : 64 * p + 64, c * 16 : (c + 1) * 16],
                        )
                    steps.append(scatter)
                    for h in (2 * p, 2 * p + 1):
                        def bmul(h=h, c=c):
                            rb = cxp.tile([HD, QCHUNK], f32, tag="ctx", name=f"rb_{c}_{h}")
                            mm(
                                rb[:],
                                ones64[32 * h : 32 * h + 1, :],
                                recip_fl[
                                    32 * h : 32 * h + 1,
                                    c * QCHUNK : (c + 1) * QCHUNK,
                                ],
                                start=True,
                                stop=True,
                                tile_position=(32 * h, 0) if h == 3 else None,
                            )
                            pp_, half = h // 2, h % 2
                            sl = ctxT[
                                half * HD : (half + 1) * HD,
                                pp_ * S + c * QCHUNK : pp_ * S + (c + 1) * QCHUNK,
                            ]
                            nc.vector.tensor_mul(sl, sl, rb[:])
                        steps.append(bmul)
                    add_chain(key, steps)

                def add_outproj(c):
                    key = ("outproj", c)
                    steps = []
                    for qt in range(4 * c, 4 * c + 4):
                        for oc in range(2):
                            def proj(qt=qt, oc=oc):
                                ps = pj.tile([128, QCHUNK], f32, tag="pj", name=f"po_{qt}_{oc}")
                                for p in range(2):
                                    mm(
                                        ps[:],
                                        ctxT[:, p * S + qt * 128 : p * S + (qt + 1) * 128],
                                        wp_sb[:, p * D + oc * QCHUNK : p * D + (oc + 1) * QCHUNK],
                                        start=(p == 0),
                                        stop=(p == 1),
                                    )
                                o = ob.tile([128, QCHUNK], bf16, tag="o")
                                nc.vector.tensor_copy(o[:], ps[:])
                                nc.sync.dma_start(
                                    out_d[
                                        qt * 128 : (qt + 1) * 128,
                                        oc * QCHUNK : (oc + 1) * QCHUNK,
                                    ],
                                    o[:],
                                )
                            steps.append(proj)
                    add_chain(key, steps)

                # ---------- main round loop ----------
                # ctx matmuls are pipeline-shifted one round late so the PE
                # queue order is [scores(r+1) | ctx(r) | fillers]: scores for
                # the next exp are never stuck behind ctx's wait-on-exp.
                for c in range(NQC):
                    for p in range(2):
                        need(("qk", p, c))
                        ctxA = cxp.tile([VW, QCHUNK], f32, tag="ctx")
                        ctxB = cxp.tile([VW, QCHUNK], f32, tag="ctx")
                        pend = None  # deferred ctx emission for previous jt

                        def emit_ctx(c=c, p=p):
                            nonlocal pend
                            if pend is None:
                                return
                            jt, e = pend
                            pend = None
                            need(("vext", jt))
                            for ctx_ps, h, half in (
                                (ctxA, 2 * p, 0),
                                (ctxB, 2 * p + 1, 1),
                            ):
                                mm(
                                    ctx_ps[:],
                                    vext[:, jt * VEXTW + h * VW : jt * VEXTW + (h + 1) * VW],
                                    e[:, half * QCHUNK : (half + 1) * QCHUNK],
                                    start=(jt == 0),
                                    stop=(jt == JT - 1),
                                    skip_group_check=True,
                                )

                        for jt in range(JT):
                            need(("qk", 2 + p, jt // 4))
                            sc = scp.tile([128, 2 * QCHUNK], f32, tag="sc")
                            for half, (lo, hi) in enumerate(((0, 64), (64, 128))):
                                mm(
                                    sc[:, half * QCHUNK : (half + 1) * QCHUNK],
                                    qkT[
                                        lo:hi,
                                        (2 + p) * S + jt * 128 : (2 + p) * S + (jt + 1) * 128,
                                    ],
                                    qkT[lo:hi, p * S + c * QCHUNK : p * S + (c + 1) * QCHUNK],
                                    start=True,
                                    stop=True,
                                )
                            e = ep.tile([128, 2 * QCHUNK], bf16, tag="e")
                            nc.scalar.activation(e[:], sc[:], Exp, scale=0.125)
                            emit_ctx()
                            pend = (jt, e)
                            pop_fillers(3)
                        emit_ctx()  # flush ctx for jt=15
                        # drain ctx rows 0-63 -> ctxT, row 64 -> denominators
                        for ctx_ps, h, half in ((ctxA, 2 * p, 0), (ctxB, 2 * p + 1, 1)):
                            nc.vector.tensor_copy(
                                ctxT[
                                    half * HD : (half + 1) * HD,
                                    p * S + c * QCHUNK : p * S + (c + 1) * QCHUNK,
                                ],
                                ctx_ps[0:HD, :],
                            )
                            nc.vector.tensor_copy(
                                sums_fl[
                                    32 * h : 32 * h + 1, c * QCHUNK : (c + 1) * QCHUNK
                                ],
                                ctx_ps[HD : HD + 1, :],
                            )
                        add_norm_pair(c, p)
                    add_outproj(c)
                # flush remaining fillers (chunk-3 norm/outproj and stragglers)
                pop_fillers(len(filler_q))

    nc.compile()
    return nc


def get_program():
    if "nc" not in _cache:
        _cache["nc"] = _build_program()
    return _cache["nc"]


def make_in_maps(x, mask, W_qkv, b_qkv, W_proj):
    """Build the 8 per-core input maps (host-side sharding)."""
    x = np.asarray(x, dtype=np.float32)
    mask = np.asarray(mask)
    W_qkv = np.asarray(W_qkv, dtype=np.float32)
    b_qkv = np.asarray(b_qkv, dtype=np.float32)
    W_proj = np.asarray(W_proj, dtype=np.float32)

    in_maps = []
    for core in range(NCORES):
        b, g = divmod(core, 4)
        qc = slice(256 * g, 256 * (g + 1))  # q cols for heads 4g..4g+3
        kc = slice(D + 256 * g, D + 256 * (g + 1))

        xT = np.ascontiguousarray(x[b].T).astype(BF16)

        wqk = np.concatenate([W_qkv[:, qc], W_qkv[:, kc]], axis=1)
        wqk = np.ascontiguousarray(wqk).astype(BF16)

        bq = b_qkv[qc]
        bk = b_qkv[kc]
        bqk = np.stack([bq[:128], bq[128:], bk[:128], bk[128:]], axis=1)  # [128, 4]
        bqk = np.ascontiguousarray(bqk)

        wv_ext = np.zeros((D, VEXTW), dtype=np.float32)
        for h in range(HEADS_PER_CORE):
            wv_ext[:, h * VW : h * VW + HD] = W_qkv[
                :, 2 * D + 256 * g + HD * h : 2 * D + 256 * g + HD * (h + 1)
            ]

        wp = np.ascontiguousarray(W_proj[256 * g : 256 * (g + 1), :]).astype(BF16)

        maskf = np.ascontiguousarray(
            mask[b].astype(np.float32).reshape(RT, 128).T
        )  # [128, RT] col t = rowtile t

        in_maps.append(
            {
                "xT": xT,
                "wqk": wqk,
                "bqk": bqk,
                "wv": wv_ext.astype(BF16),
                "wp": wp,
                "maskf": maskf,
                "ones64": np.ones((128, 64), dtype=np.float32),
            }
        )
    return in_maps


def kernel(x, mask, W_qkv, b_qkv, W_proj, b_proj, _trace=False):
    from concourse import bass_utils

    nc = get_program()
    in_maps = make_in_maps(x, mask, W_qkv, b_qkv, W_proj)

    res = bass_utils.run_bass_kernel_spmd(
        nc, in_maps, list(range(NCORES)), trace=_trace
    )
    _cache["last_results"] = res

    b_qkv = np.asarray(b_qkv, dtype=np.float32)
    W_proj = np.asarray(W_proj, dtype=np.float32)
    bias_full = np.asarray(b_proj, dtype=np.float32) + b_qkv[2 * D :] @ W_proj

    out = np.empty((B, S, D), dtype=np.float32)
    for b in range(B):
        acc = bias_full[None, :].repeat(S, axis=0).astype(np.float32)
        for g in range(4):
            acc = acc + np.asarray(res.results[b * 4 + g]["out"]).astype(np.float32)
        out[b] = acc
    return out


# revision 10
# speedup vs baseline: 1.6147x; 1.0085x over previous
"""
Multi-head attention (B=2, S=2048, D=1024, H=16, hd=64) on 8 TRN2 NeuronCores.

Sharding: tensor-parallel over (batch, head-group).
  core = b*4 + g   (b in {0,1}, g in {0..3})  owns batch b, heads 4g..4g+3.

v2 pipeline design (vs baseline): the scalar engine (exp) is the roofline at
~1.15us per [128,1024] round; everything else is scheduled to hide under it.

  - Rounds of 1 j-tile x 1 head-pair: scores psum [128,1024] double-buffered
    (2x2 banks) so exp(r) overlaps scores(r+1); exp runs back-to-back.
  - ctx psum [65,512] per head accumulates over the 16 j-tiles of a
    (chunk, pair); row 64 rides the matmul as the softmax denominator
    (vext mask-column trick, unchanged from baseline).
  - Chunk-outer loop: after both pairs finish chunk c, normalize + output
    projection + DMA for chunk c are emitted as fillers into chunk c+1's
    rounds (no serial tail except chunk 3).
  - Projection chains (qT/kT per (ptile,chunk), vext per row-tile) are
    emitted JIT via a filler queue: Tile's scheduler priority == emission
    order, so filler closures are popped between rounds, with forced
    emission of any chain a round depends on.
  - PSUM budget: sc 2x2 + ctx/rb 2 + proj 2 = 8 banks.
  - Output partials stored bf16 (halves DMA; host sums in f32).
"""

import ml_dtypes
import numpy as np

BF16 = ml_dtypes.bfloat16

B, S, D = 2, 2048, 1024
H, HD = 16, 64
NCORES = 8
HEADS_PER_CORE = 4
KSLICES = D // 128  # 8
QCHUNK = 512
NQC = S // QCHUNK  # 4
JT = S // 128  # 16 j tiles
RT = S // 128  # 16 row tiles
VW = HD + 1  # 65: v columns + mask column
VEXTW = HEADS_PER_CORE * VW  # 260

_cache = {}


def _build_program():
    import concourse.bass as bass
    import concourse.tile as tile
    from concourse import bacc, mybir

    f32 = mybir.dt.float32
    f32r = mybir.dt.float32r
    bf16 = mybir.dt.bfloat16
    Exp = mybir.ActivationFunctionType.Exp

    nc = bacc.Bacc(
        "TRN2",
        target_bir_lowering=False,
        debug=False,
        num_devices=NCORES,
        enable_partition_id=False,
    )

    xT_d = nc.dram_tensor("xT", [D, S], bf16, kind="ExternalInput").ap()
    wqk_d = nc.dram_tensor("wqk", [D, 512], bf16, kind="ExternalInput").ap()
    bqk_d = nc.dram_tensor("bqk", [128, 4], f32, kind="ExternalInput").ap()
    wv_d = nc.dram_tensor("wv", [D, VEXTW], bf16, kind="ExternalInput").ap()
    wp_d = nc.dram_tensor("wp", [256, D], bf16, kind="ExternalInput").ap()
    maskf_d = nc.dram_tensor("maskf", [128, RT], f32, kind="ExternalInput").ap()
    ones64_d = nc.dram_tensor("ones64", [128, 64], f32r, kind="ExternalInput").ap()
    out_d = nc.dram_tensor("out", [S, D], bf16, kind="ExternalOutput").ap()

    def mm(out, lhsT, rhs, **kw):
        nc.tensor.matmul(out, lhsT, rhs, **kw)

    with tile.TileContext(nc) as tc:
        with tc.tile_pool(name="persist", bufs=1) as pp:
            qkT = pp.tile([128, 4 * S], bf16, tag="qkT")
            vext = pp.tile([128, RT * VEXTW], bf16, tag="vext")
            wp_sb = pp.tile([128, 2 * D], bf16, tag="wp")
            maskf = pp.tile([128, RT], f32, tag="maskf")
            bqk = pp.tile([128, 4], f32, tag="bqk")
            ones4 = pp.tile([128, 4], f32, tag="ones4")
            ctxT = pp.tile([128, 2 * S], bf16, tag="ctxT")
            ones64 = pp.tile([128, 64], f32r, tag="ones64")
            sums_fl = pp.tile([128, S], f32, tag="sums_fl")
            recip_fl = pp.tile([128, S], f32r, tag="recip_fl")
            sums_rs = pp.tile([128, 64], f32, tag="sums_rs")
            recip_rs = pp.tile([128, 64], f32r, tag="recip_rs")
            xT = pp.tile([128, KSLICES * S], bf16, tag="xT")
            wqk = pp.tile([128, KSLICES * 512], bf16, tag="wqk")
            wv = pp.tile([128, KSLICES * VEXTW], bf16, tag="wv")

            # ---- input DMAs, ordered by first use; xT split into column
            # halves so chunk-0/strip-0 chains unblock at ~60% of the load ----
            nc.gpsimd.memset(ones4[:], 1.0)
            nc.gpsimd.memset(sums_fl[:], 1.0)
            for k in range(KSLICES):
                nc.sync.dma_start(
                    wqk[:, k * 512 : (k + 1) * 512], wqk_d[k * 128 : (k + 1) * 128, :]
                )
            for k in range(KSLICES):
                nc.gpsimd.dma_start(
                    wv[:, k * VEXTW : (k + 1) * VEXTW],
                    wv_d[k * 128 : (k + 1) * 128, :],
                )
            nc.sync.dma_start(bqk[:], bqk_d[:])
            nc.sync.dma_start(maskf[:], maskf_d[:])
            for k in range(KSLICES):
                nc.scalar.dma_start(
                    xT[:, k * S : k * S + 1024],
                    xT_d[k * 128 : (k + 1) * 128, 0:1024],
                )
            for k in range(KSLICES):
                nc.gpsimd.dma_start(
                    xT[:, k * S + 1024 : (k + 1) * S],
                    xT_d[k * 128 : (k + 1) * 128, 1024:2048],
                )
            nc.sync.dma_start(ones64[:], ones64_d[:])
            for p in range(2):
                nc.sync.dma_start(
                    wp_sb[:, p * D : (p + 1) * D], wp_d[p * 128 : (p + 1) * 128, :]
                )

            with (
                tc.tile_pool(name="pj", bufs=2, space="PSUM") as pj,
                tc.tile_pool(name="sc", bufs=2, space="PSUM") as scp,
                tc.tile_pool(name="cx", bufs=2, space="PSUM") as cxp,
                tc.tile_pool(name="ep", bufs=3) as ep,
                tc.tile_pool(name="ob", bufs=2) as ob,
            ):
                # ---------- filler machinery ----------
                # Each chain is a list of closures; emission order == Tile
                # scheduling priority, so chains are dribbled between rounds.
                filler_q = []  # list of (key, closure)
                done = set()  # chain keys fully emitted
                emitted_keys = set()

                def make_qk_chain(pt, c):
                    st = {}
                    steps = []
                    for k in range(KSLICES):
                        def step(k=k, pt=pt, c=c):
                            if k == 0:
                                st["ps"] = pj.tile([128, QCHUNK], f32, tag="pj", name=f"pjqk_{pt}_{c}")
                            mm(
                                st["ps"][:],
                                wqk[:, k * 512 + pt * 128 : k * 512 + (pt + 1) * 128],
                                xT[:, k * S + c * QCHUNK : k * S + (c + 1) * QCHUNK],
                                start=(k == 0),
                                stop=(k == KSLICES - 1),
                            )
                        steps.append(step)
                    def drain(pt=pt, c=c):
                        nc.vector.tensor_scalar_add(
                            qkT[:, pt * S + c * QCHUNK : pt * S + (c + 1) * QCHUNK],
                            st["ps"][:],
                            bqk[:, pt : pt + 1],
                        )
                    steps.append(drain)
                    return steps

                def make_v_chain(t):
                    st = {}
                    steps = []
                    for k in range(KSLICES):
                        def step(k=k, t=t):
                            if k == 0:
                                st["ps"] = pj.tile([128, VEXTW], f32, tag="pj", name=f"pjv_{t}")
                            mm(
                                st["ps"][:],
                                xT[:, k * S + t * 128 : k * S + (t + 1) * 128],
                                wv[:, k * VEXTW : (k + 1) * VEXTW],
                                start=(k == 0),
                                stop=(k == KSLICES - 1),
                            )
                        steps.append(step)
                    def drain(t=t):
                        sl = vext[:, t * VEXTW : (t + 1) * VEXTW]
                        nc.vector.tensor_scalar_mul(sl, st["ps"][:], maskf[:, t : t + 1])
                        mcols = sl.rearrange("p (h w) -> p h w", w=VW)[:, :, HD]
                        nc.vector.tensor_scalar_mul(
                            mcols, ones4[:, 0:4], maskf[:, t : t + 1]
                        )
                    steps.append(drain)
                    return steps

                def add_chain(key, steps):
                    emitted_keys.add(key)
                    for s in steps:
                        filler_q.append((key, s))

                # chains ordered by first use:
                # round index = c*32 + p*16 + jt
                chain_first_use = []
                for p in range(2):
                    for c in range(NQC):
                        chain_first_use.append((c * 32 + p * 16, ("qk", p, c)))
                    for s_ in range(4):
                        chain_first_use.append((p * 16 + 4 * s_, ("qk", 2 + p, s_)))
                for t in range(RT):
                    chain_first_use.append((t, ("vext", t)))
                chain_first_use.sort(key=lambda x: x[0])
                for _, key in chain_first_use:
                    if key[0] == "qk":
                        add_chain(key, make_qk_chain(key[1], key[2]))
                    else:
                        add_chain(key, make_v_chain(key[1]))

                def need(key):
                    if key in done:
                        return
                    while key not in done:
                        k2, closure = filler_q.pop(0)
                        closure()
                        if not filler_q or filler_q[0][0] != k2:
                            done.add(k2)

                def pop_fillers(n):
                    for _ in range(n):
                        if not filler_q:
                            return
                        k2, closure = filler_q.pop(0)
                        closure()
                        if not filler_q or filler_q[0][0] != k2:
                            done.add(k2)

                # ---------- norm + output projection fillers (per chunk) ----------
                def add_norm_pair(c, p):
                    """Normalize pair p's 2 heads of chunk c (overlaps other work)."""
                    key = ("norm", c, p)
                    steps = []
                    def gather(c=c, p=p):
                        nc.gpsimd.dma_start(
                            sums_rs[64 * p : 64 * p + 64, c * 16 : (c + 1) * 16],
                            sums_fl[
                                64 * p : 64 * p + 64, c * QCHUNK : (c + 1) * QCHUNK
                            ].rearrange("(a b) f -> a b f", b=32)[:, 0, :],
                        )
                    steps.append(gather)
                    def recip(c=c, p=p):
                        with nc.allow_low_precision(reason="f32r softmax recip"):
                            nc.vector.reciprocal(
                                recip_rs[64 * p : 64 * p + 64, c * 16 : (c + 1) * 16],
                                sums_rs[64 * p : 64 * p + 64, c * 16 : (c + 1) * 16],
                            )
                    steps.append(recip)
                    def scatter(c=c, p=p):
                        nc.gpsimd.dma_start(
                            recip_fl[
                                64 * p : 64 * p + 64, c * QCHUNK : (c + 1) * QCHUNK
                            ].rearrange("(a b) f -> a b f", b=32)[:, 0, :],
                            recip_rs[64 * p : 64 * p + 64, c * 16 : (c + 1) * 16],
                        )
                    steps.append(scatter)
                    for h in (2 * p, 2 * p + 1):
                        def bmul(h=h, c=c):
                            rb = pj.tile([HD, QCHUNK], f32, tag="pj", name=f"rb_{c}_{h}")
                            mm(
                                rb[:],
                                ones64[32 * h : 32 * h + 1, :],
                                recip_fl[
                                    32 * h : 32 * h + 1,
                                    c * QCHUNK : (c + 1) * QCHUNK,
                                ],
                                start=True,
                                stop=True,
                                tile_position=(32 * h, 0) if h == 3 else None,
                            )
                            pp_, half = h // 2, h % 2
                            sl = ctxT[
                                half * HD : (half + 1) * HD,
                                pp_ * S + c * QCHUNK : pp_ * S + (c + 1) * QCHUNK,
                            ]
                            nc.vector.tensor_mul(sl, sl, rb[:])
                        steps.append(bmul)
                    add_chain(key, steps)

                def add_outproj(c):
                    key = ("outproj", c)
                    steps = []
                    last = c == NQC - 1
                    for i, (qt, oc) in enumerate(
                        (qt, oc) for qt in range(4 * c, 4 * c + 4) for oc in range(2)
                    ):
                        if True:
                            def proj(qt=qt, oc=oc, i=i, last=last):
                                pool = scp if (last and i % 2) else pj
                                ps = pool.tile(
                                    [128, QCHUNK], f32,
                                    tag="sc" if (last and i % 2) else "pj",
                                    name=f"po_{qt}_{oc}",
                                )
                                for p in range(2):
                                    mm(
                                        ps[:],
                                        ctxT[:, p * S + qt * 128 : p * S + (qt + 1) * 128],
                                        wp_sb[:, p * D + oc * QCHUNK : p * D + (oc + 1) * QCHUNK],
                                        start=(p == 0),
                                        stop=(p == 1),
                                    )
                                o = ob.tile([128, QCHUNK], bf16, tag="o")
                                if last:
                                    nc.scalar.copy(o[:], ps[:])
                                else:
                                    nc.vector.tensor_copy(o[:], ps[:])
                                nc.sync.dma_start(
                                    out_d[
                                        qt * 128 : (qt + 1) * 128,
                                        oc * QCHUNK : (oc + 1) * QCHUNK,
                                    ],
                                    o[:],
                                )
                            steps.append(proj)
                    add_chain(key, steps)

                # warm the queue: force the first round's deps, then pull
                # filler chains forward into the DMA ramp window
                need(("qk", 0, 0))
                need(("qk", 2, 0))
                pop_fillers(20)

                # ---------- main round loop ----------
                # ctx matmuls are pipeline-shifted one round late so the PE
                # queue order is [scores(r+1) | ctx(r) | fillers]: scores for
                # the next exp are never stuck behind ctx's wait-on-exp.
                for c in range(NQC):
                    for p in range(2):
                        need(("qk", p, c))
                        ctxA = cxp.tile([VW, QCHUNK], f32, tag="ctx")
                        ctxB = cxp.tile([VW, QCHUNK], f32, tag="ctx")
                        pend = None  # deferred ctx emission for previous jt

                        def emit_ctx(c=c, p=p):
                            nonlocal pend
                            if pend is None:
                                return
                            jt, e = pend
                            pend = None
                            need(("vext", jt))
                            for ctx_ps, h, half in (
                                (ctxA, 2 * p, 0),
                                (ctxB, 2 * p + 1, 1),
                            ):
                                mm(
                                    ctx_ps[:],
                                    vext[:, jt * VEXTW + h * VW : jt * VEXTW + (h + 1) * VW],
                                    e[:, half * QCHUNK : (half + 1) * QCHUNK],
                                    start=(jt == 0),
                                    stop=(jt == JT - 1),
                                    skip_group_check=True,
                                )

                        for jt in range(JT):
                            need(("qk", 2 + p, jt // 4))
                            sc = scp.tile([128, 2 * QCHUNK], f32, tag="sc")
                            for half, (lo, hi) in enumerate(((0, 64), (64, 128))):
                                mm(
                                    sc[:, half * QCHUNK : (half + 1) * QCHUNK],
                                    qkT[
                                        lo:hi,
                                        (2 + p) * S + jt * 128 : (2 + p) * S + (jt + 1) * 128,
                                    ],
                                    qkT[lo:hi, p * S + c * QCHUNK : p * S + (c + 1) * QCHUNK],
                                    start=True,
                                    stop=True,
                                )
                            e = ep.tile([128, 2 * QCHUNK], bf16, tag="e")
                            nc.scalar.activation(e[:], sc[:], Exp, scale=0.125)
                            emit_ctx()
                            pend = (jt, e)
                            pop_fillers(3)
                        emit_ctx()  # flush ctx for jt=15
                        # drain ctx rows 0-63 -> ctxT, row 64 -> denominators
                        # (last block: scalar engine -- ACT is idle by then)
                        tcopy = (
                            nc.scalar.copy
                            if (c == NQC - 1 and p == 1)
                            else nc.vector.tensor_copy
                        )
                        for ctx_ps, h, half in ((ctxA, 2 * p, 0), (ctxB, 2 * p + 1, 1)):
                            tcopy(
                                ctxT[
                                    half * HD : (half + 1) * HD,
                                    p * S + c * QCHUNK : p * S + (c + 1) * QCHUNK,
                                ],
                                ctx_ps[0:HD, :],
                            )
                            tcopy(
                                sums_fl[
                                    32 * h : 32 * h + 1, c * QCHUNK : (c + 1) * QCHUNK
                                ],
                                ctx_ps[HD : HD + 1, :],
                            )
                        add_norm_pair(c, p)
                    add_outproj(c)
                # flush remaining fillers (chunk-3 norm/outproj and stragglers)
                pop_fillers(len(filler_q))

    nc.compile()
    return nc


def get_program():
    if "nc" not in _cache:
        _cache["nc"] = _build_program()
    return _cache["nc"]


def make_in_maps(x, mask, W_qkv, b_qkv, W_proj):
    """Build the 8 per-core input maps (host-side sharding)."""
    x = np.asarray(x, dtype=np.float32)
    mask = np.asarray(mask)
    W_qkv = np.asarray(W_qkv, dtype=np.float32)
    b_qkv = np.asarray(b_qkv, dtype=np.float32)
    W_proj = np.asarray(W_proj, dtype=np.float32)

    in_maps = []
    for core in range(NCORES):
        b, g = divmod(core, 4)
        qc = slice(256 * g, 256 * (g + 1))  # q cols for heads 4g..4g+3
        kc = slice(D + 256 * g, D + 256 * (g + 1))

        xT = np.ascontiguousarray(x[b].T).astype(BF16)

        wqk = np.concatenate([W_qkv[:, qc], W_qkv[:, kc]], axis=1)
        wqk = np.ascontiguousarray(wqk).astype(BF16)

        bq = b_qkv[qc]
        bk = b_qkv[kc]
        bqk = np.stack([bq[:128], bq[128:], bk[:128], bk[128:]], axis=1)  # [128, 4]
        bqk = np.ascontiguousarray(bqk)

        wv_ext = np.zeros((D, VEXTW), dtype=np.float32)
        for h in range(HEADS_PER_CORE):
            wv_ext[:, h * VW : h * VW + HD] = W_qkv[
                :, 2 * D + 256 * g + HD * h : 2 * D + 256 * g + HD * (h + 1)
            ]

        wp = np.ascontiguousarray(W_proj[256 * g : 256 * (g + 1), :]).astype(BF16)

        maskf = np.ascontiguousarray(
            mask[b].astype(np.float32).reshape(RT, 128).T
        )  # [128, RT] col t = rowtile t

        in_maps.append(
            {
                "xT": xT,
                "wqk": wqk,
                "bqk": bqk,
                "wv": wv_ext.astype(BF16),
                "wp": wp,
                "maskf": maskf,
                "ones64": np.ones((128, 64), dtype=np.float32),
            }
        )
    return in_maps


def kernel(x, mask, W_qkv, b_qkv, W_proj, b_proj, _trace=False):
    from concourse import bass_utils

    nc = get_program()
    in_maps = make_in_maps(x, mask, W_qkv, b_qkv, W_proj)

    res = bass_utils.run_bass_kernel_spmd(
        nc, in_maps, list(range(NCORES)), trace=_trace
    )
    _cache["last_results"] = res

    b_qkv = np.asarray(b_qkv, dtype=np.float32)
    W_proj = np.asarray(W_proj, dtype=np.float32)
    bias_full = np.asarray(b_proj, dtype=np.float32) + b_qkv[2 * D :] @ W_proj

    out = np.empty((B, S, D), dtype=np.float32)
    for b in range(B):
        acc = bias_full[None, :].repeat(S, axis=0).astype(np.float32)
        for g in range(4):
            acc = acc + np.asarray(res.results[b * 4 + g]["out"]).astype(np.float32)
        out[b] = acc
    return out
